# revision 26
# baseline (speedup 1.0000x reference)
"""Trainium2 Bass kernel for nn_Net_79027398246747 (4-layer binarized MLP).

Strategy (v2 — all-fp8 DoubleRow):
- Data-parallel over batch: 8 cores x 512 rows each; weights replicated.
- Feature-major internal layout (h.T [F, B]).
- Layer 1: x is decomposed on the host into SIX exact 4-bit nibble fields
  (24-bit fixed point of x, |x| < 8, LSB 2^-21, RNE at the bottom). Each
  nibble field times the +-1 weights is an exact fp8e4m3 DoubleRow matmul
  (field values n*2^s are exactly representable, incl. subnormals -
  verified bitwise on HW). Fields 0-2 accumulate at true scale in PSUM
  group A; fields 3-5 at 2^12 x true scale in group B. The combine
  u = B*2^-12 + A is one DVE scalar_tensor_tensor op. This runs L1 at
  3 cycles/row/column instead of the fp32 path's 4 and the fp32r hi/lo
  path's 2 passes (measured: fp32r keeps only 12 mantissa bits per
  operand, so 2x12 bits was the minimum there; fp8-DR carries 4 bits per
  0.5-cycle pass = 25% fewer PE cycles for the same 24 bits).
- Layers 2-4: activations +-0.5, weights +-1 -> exact fp8e4 DoubleRow.
- Epilogues are fused: BN gamma>0 and rsqrt>0, so sign(BN(p)) == (p >= T)
  with per-feature threshold T = m - b/(g*r) precomputed on the host
  (halved for layers fed by +-0.5 activations). One DVE op per f-tile:
  h = (p >= T) - 0.5; the missing 2x is folded into the next layer's
  threshold/affine. Layer 4 keeps the fused affine y = p*A + C.
- npasses=4 averaging: passes identical; replicate XLA's CSE'd tail
  (((y+y)+y)+y)*0.25 bitwise on host.
"""

import numpy as np
import ml_dtypes

B, IN, H, OUT = 4096, 3072, 4096, 1000
OUTP = 1024            # padded output features
NCORES = 8
NB = B // NCORES       # 512 batch rows per core
K1 = IN // 128         # 24 K-chunks for layer 1
K2 = H // 128          # 32 K-chunks for layers 2-4
F1 = H // 128          # 32 output feature tiles for layers 1-3
F4 = OUTP // 128       # 8 output feature tiles for layer 4
NI = 4                 # L1 f-tiles processed field-interleaved at the start
EPS = np.float32(1e-5)

_CACHE = {}


def _build_bass():
    import concourse.bacc as bacc
    import concourse.mybir as mybir
    from concourse.tile import TileContext

    fp32 = mybir.dt.float32
    fp8 = mybir.dt.float8e4
    DR = mybir.MatmulPerfMode.DoubleRow
    AO = mybir.AluOpType

    nc = bacc.Bacc(trn_type="TRN2")

    xf = nc.dram_tensor("xf", [6, 128, K1, NB], fp8, kind="ExternalInput")
    w1 = nc.dram_tensor("w1", [F1, 128, K1, 128], fp8, kind="ExternalInput")
    w2 = nc.dram_tensor("w2", [F1, 128, K2, 128], fp8, kind="ExternalInput")
    w3 = nc.dram_tensor("w3", [F1, 128, K2, 128], fp8, kind="ExternalInput")
    w4 = nc.dram_tensor("w4", [F4, 128, K2, 128], fp8, kind="ExternalInput")
    # Per-feature thresholds [128, 3, F1]: rows (T1, T2, T3), feature=f*128+p
    thr = nc.dram_tensor("thr", [128, 3, F1], fp32, kind="ExternalInput")
    bn4 = nc.dram_tensor("bn4", [128, 2, F4], fp32, kind="ExternalInput")
    out = nc.dram_tensor("out", [OUTP, NB], fp32, kind="ExternalOutput")

    with TileContext(nc) as tc:
        with tc.tile_pool(name="persist", bufs=1) as persist, \
             tc.tile_pool(name="w1pool", bufs=4) as w1pool, \
             tc.tile_pool(name="w23pool", bufs=8) as w23pool, \
             tc.tile_pool(name="zpool", bufs=3) as zpool, \
             tc.tile_pool(name="hpool", bufs=2) as hpool, \
             tc.tile_pool(name="psum", bufs=8, space="PSUM") as psum:

            xft = persist.tile([128, 6, K1, NB], fp8, name="xft")
            thrt = persist.tile([128, 3, F1], fp32)
            bnt4 = persist.tile([128, 2, F4], fp32)

            # --- DMA lead-in ---
            # x streams in 4-chunk pieces (728 ns transfer > the 625 ns
            # serialized HWDGE descriptor-gen, so the stream is not
            # HWDGE-bound). Order: w1[0], first 3 pieces, w1[1..3], rest.
            NP = 6 * K1 // 4                   # 36 pieces, 6 per field

            def dma_piece(q):
                i, t = divmod(q, 6)
                nc.sync.dma_start(out=xft[:, i, 4 * t:4 * t + 4, :],
                                  in_=xf[i, :, 4 * t:4 * t + 4, :])

            w1_pre = []
            w8_0 = w1pool.tile([128, K1, 128], fp8, tag="w1", name="w1_0")
            nc.sync.dma_start(out=w8_0[:], in_=w1[0])
            w1_pre.append(w8_0)
            for q in range(3):
                dma_piece(q)
            for f in range(1, NI):
                w8 = w1pool.tile([128, K1, 128], fp8, tag=f"w1p{f}",
                                 name=f"w1_{f}")
                nc.sync.dma_start(out=w8[:], in_=w1[f])
                w1_pre.append(w8)
            for q in range(3, NP):
                dma_piece(q)
            # needed first at the phase-A epilogues, well after the stream
            nc.sync.dma_start(out=thrt[:], in_=thr[:])
            nc.sync.dma_start(out=bnt4[:], in_=bn4[:])

            # h1 and h3 share a buffer (bufs=2 ring on one tag): h1 is fully
            # consumed by layer 2 before layer 3's epilogue writes h3.
            hs = [hpool.tile([128, F1, NB], fp8, tag="h", name=f"h{i}")
                  for i in range(3)]

            def epi1(psA, psB, f):
                # t = B*2^-12 (ACT, PSUM->SBUF); u = (A - T1) + t (DVE);
                # h = (u >= 0) - 0.5 (DVE)
                t = zpool.tile([128, NB], fp32, tag="t")
                nc.scalar.activation(t[:], psB[:],
                                     mybir.ActivationFunctionType.Copy,
                                     bias=0.0, scale=float(2.0 ** -12))
                u = zpool.tile([128, NB], fp32, tag="u")
                nc.vector.scalar_tensor_tensor(u[:], psA[:],
                                               thrt[:, 0, f:f + 1], t[:],
                                               op0=AO.subtract, op1=AO.add)
                nc.vector.tensor_scalar(hs[0][:, f, :], u[:], 0.0, 0.5,
                                        op0=AO.is_ge, op1=AO.subtract)

            # ---- Layer 1: 6 nibble-field fp8 DR passes, 2 PSUM groups ----
            # Phase A: first NI f-tiles run piece-interleaved so the PE
            # consumes each landed x piece NI times immediately. Emission
            # order tracks the DMA stream: pieces 0-2 tile-major (tile 0 can
            # start before w1[1..3] land), then piece-major for the rest.
            psA = [psum.tile([128, NB], fp32, tag="ps", name=f"psA{f}")
                   for f in range(NI)]
            psB = [psum.tile([128, NB], fp32, tag="ps", name=f"psB{f}")
                   for f in range(NI)]

            def l1mm(ps_pair, w8, q):
                # piece q = (field i, quarter t) -> 2 DR matmuls.
                # Group A = pieces 0..17 (fields 0-2), B = 18..35.
                i, t = divmod(q, 6)
                ps = ps_pair[0] if i < 3 else ps_pair[1]
                for j in (2 * t, 2 * t + 1):
                    nc.tensor.matmul(ps[:], w8[:, 2 * j:2 * j + 2, :],
                                     xft[:, i, 2 * j:2 * j + 2, :],
                                     start=(q % 18 == 0 and j == 2 * t),
                                     stop=(q % 18 == 17 and j == 2 * t + 1),
                                     perf_mode=DR)

            for f in range(NI):
                for q in range(3):
                    l1mm((psA[f], psB[f]), w1_pre[f], q)
            for q in range(3, NP):
                for f in range(NI):
                    l1mm((psA[f], psB[f]), w1_pre[f], q)
            for f in range(NI):
                epi1(psA[f], psB[f], f)

            # Phase B: remaining f-tiles, sequential
            for f in range(NI, F1):
                w8 = w1pool.tile([128, K1, 128], fp8, tag="w1")
                nc.sync.dma_start(out=w8[:], in_=w1[f])
                pa = psum.tile([128, NB], fp32, tag="ps")
                pb = psum.tile([128, NB], fp32, tag="ps")
                for i in range(6):
                    ps = pa if i < 3 else pb
                    ii = i % 3
                    for j in range(K1 // 2):
                        nc.tensor.matmul(ps[:],
                                         w8[:, 2 * j:2 * j + 2, :],
                                         xft[:, i, 2 * j:2 * j + 2, :],
                                         start=(ii == 0 and j == 0),
                                         stop=(ii == 2 and j == K1 // 2 - 1),
                                         perf_mode=DR)
                epi1(pa, pb, f)

            # Prefetch the first w4 tile early so layer 4 starts fed.
            w4pre = persist.tile([128, K2, 128], fp8, name="w4pre")
            nc.sync.dma_start(out=w4pre[:], in_=w4[0])

            # ---- Layers 2-3: fp8 DR, fused threshold epilogue ----
            for li, (w, hin, hout) in enumerate([(w2, hs[0], hs[1]),
                                                 (w3, hs[1], hs[2])], start=1):
                for f in range(F1):
                    w8 = w23pool.tile([128, K2, 128], fp8, tag="w23")
                    if f == 0:
                        nc.sync.dma_start(out=w8[:, :K2 // 2, :],
                                          in_=w[f, :, :K2 // 2, :])
                        nc.sync.dma_start(out=w8[:, K2 // 2:, :],
                                          in_=w[f, :, K2 // 2:, :])
                    else:
                        nc.sync.dma_start(out=w8[:], in_=w[f])
                    ps = psum.tile([128, NB], fp32, tag="ps")
                    for i in range(K2 // 2):
                        nc.tensor.matmul(ps[:], w8[:, 2 * i:2 * i + 2, :],
                                         hin[:, 2 * i:2 * i + 2, :],
                                         start=(i == 0),
                                         stop=(i == K2 // 2 - 1),
                                         perf_mode=DR)
                    nc.vector.tensor_scalar(hout[:, f, :], ps[:],
                                            thrt[:, li, f:f + 1], 0.5,
                                            op0=AO.is_ge, op1=AO.subtract)

            # ---- Layer 4: fp8 DR + fused affine y = p*A + C ----
            # The last tile runs as two half-batch PSUM groups so the final
            # epilogue + store overlap the closing matmuls.
            def epi4(ps, f, sl, eng=None):
                n = sl.stop - sl.start
                y = zpool.tile([128, n], fp32, tag="y")
                nc.vector.tensor_scalar(y[:], ps[:], bnt4[:, 0, f:f + 1],
                                        bnt4[:, 1, f:f + 1],
                                        op0=AO.mult, op1=AO.add)
                (eng or nc.sync).dma_start(out=out[f * 128:(f + 1) * 128, sl],
                                           in_=y[:])

            for f in range(F4):
                if f == 0:
                    w8 = w4pre
                else:
                    w8 = w23pool.tile([128, K2, 128], fp8, tag="w23",
                                      name=f"w4t{f}")
                    nc.sync.dma_start(out=w8[:], in_=w4[f])
                if f < F4 - 1:
                    ps = psum.tile([128, NB], fp32, tag="ps")
                    for i in range(K2 // 2):
                        nc.tensor.matmul(ps[:], w8[:, 2 * i:2 * i + 2, :],
                                         hs[2][:, 2 * i:2 * i + 2, :],
                                         start=(i == 0),
                                         stop=(i == K2 // 2 - 1),
                                         perf_mode=DR)
                    epi4(ps, f, slice(0, NB))
                else:
                    for half in range(2):
                        sl = slice(half * (NB // 2), (half + 1) * (NB // 2))
                        ph = psum.tile([128, NB // 2], fp32, tag="ps")
                        for i in range(K2 // 2):
                            nc.tensor.matmul(ph[:], w8[:, 2 * i:2 * i + 2, :],
                                             hs[2][:, 2 * i:2 * i + 2, sl],
                                             start=(i == 0),
                                             stop=(i == K2 // 2 - 1),
                                             perf_mode=DR)
                        epi4(ph, f, sl)

    nc.finalize()
    return nc


def _device_rsqrt(v):
    """rsqrt(v + eps) with the same bits as the neuron reference."""
    import jax
    fn = _CACHE.get("rsqrt_fn")
    if fn is None:
        fn = jax.jit(lambda t: jax.lax.rsqrt(t + EPS))
        _CACHE["rsqrt_fn"] = fn
    return np.asarray(fn(v.astype(np.float32)))


def _sign8(w):
    return np.where(w >= 0, 1, -1).astype(ml_dtypes.float8_e4m3)


def _prep_w(ws, n_k, n_f):
    # ws: [F_total, K_total] +-1 fp8 -> [n_f, 128, n_k, 128]:
    # out[f, p, k, j] = ws[f*128+j, k*128+p]
    a = ws.reshape(n_f, 128, n_k, 128)          # [f, j, k, p]
    return np.ascontiguousarray(a.transpose(0, 3, 2, 1))


def _threshold(bn, half):
    # bn: [4, F] (g, b, m, v); sign(BN(S)) == (S >= T), T = m - b/(g*r).
    # half: incoming activations are +-0.5 (preact = S/2) -> T/2.
    g, b, m, v = bn[0], bn[1], bn[2], bn[3]
    r = _device_rsqrt(v)
    T = (m - b / (g * r)).astype(np.float32)
    if half:
        T = (T * np.float32(0.5)).astype(np.float32)
    return T


def _nibble_fields(x):
    # x [B, IN] fp32, |x| < 8 -> 6 stored-scale nibble field arrays fp8.
    # |X| = rint(|x| * 2^21) = sum_i n_i * 2^(20-4i); true scale of field i
    # is 2^(-1-4i); fields 3-5 stored at 2^12 x true (group B).
    X = np.rint(x * np.float32(2.0 ** 21)).astype(np.int32)
    s = np.sign(X).astype(np.int8)
    a = np.minimum(np.abs(X), (1 << 24) - 1)   # clamp |x| to <8 (randn: never)
    fields = []
    for i in range(6):
        nib = ((a >> (20 - 4 * i)) & 0xF).astype(np.float32)
        scale = np.float32(2.0 ** (-1 - 4 * (i % 3)))
        f = (nib * scale) * s
        fields.append(f.astype(ml_dtypes.float8_e4m3))
    return fields


def kernel(x, w1, w2, w3, w4, bn1, bn2, bn3, bn4):
    from concourse.bass_utils import run_bass_kernel_spmd

    x = np.asarray(x, dtype=np.float32)
    nc = _CACHE.get("nc")
    if nc is None:
        nc = _build_bass()
        _CACHE["nc"] = nc

    w1p = _prep_w(_sign8(np.asarray(w1)), K1, F1)
    w2p = _prep_w(_sign8(np.asarray(w2)), K2, F1)
    w3p = _prep_w(_sign8(np.asarray(w3)), K2, F1)
    w4s = _sign8(np.asarray(w4))
    w4pad = np.zeros((OUTP, H), dtype=ml_dtypes.float8_e4m3)
    w4pad[:OUT] = w4s
    w4p = _prep_w(w4pad, K2, F4)

    # thresholds [128, 3, F1]: thr[p, l, f] = T_l[f*128+p]
    T1 = _threshold(np.asarray(bn1), half=False)
    T2 = _threshold(np.asarray(bn2), half=True)
    T3 = _threshold(np.asarray(bn3), half=True)
    thr = np.stack([T1, T2, T3]).reshape(3, F1, 128)
    thr = np.ascontiguousarray(thr.transpose(2, 0, 1)).astype(np.float32)

    # layer-4 fused affine with the 0.5 fold: A = 2*g*r, C = b - (m*g)*r
    bn4a = np.asarray(bn4)
    g4 = np.zeros(OUTP, np.float32)
    b4 = np.zeros(OUTP, np.float32)
    m4 = np.zeros(OUTP, np.float32)
    v4 = np.full(OUTP, 1.0, np.float32)
    g4[:OUT], b4[:OUT], m4[:OUT], v4[:OUT] = bn4a[0], bn4a[1], bn4a[2], bn4a[3]
    r4 = _device_rsqrt(v4)
    A4 = (np.float32(2.0) * g4 * r4).astype(np.float32)
    C4 = (b4 - (m4 * g4) * r4).astype(np.float32)
    b4t = np.stack([A4, C4]).reshape(2, F4, 128)
    b4t = np.ascontiguousarray(b4t.transpose(2, 0, 1)).astype(np.float32)

    fields = _nibble_fields(x)

    in_maps = []
    for c in range(NCORES):
        # field layout [128, K1, NB]: [p, k, n] = field[c*NB+n, k*128+p]
        xfc = np.empty((6, 128, K1, NB), dtype=ml_dtypes.float8_e4m3)
        for i in range(6):
            sl = fields[i][c * NB:(c + 1) * NB]           # [NB, IN]
            xfc[i] = sl.reshape(NB, K1, 128).transpose(2, 1, 0)
        in_maps.append({"xf": xfc, "w1": w1p, "w2": w2p, "w3": w3p,
                        "w4": w4p, "thr": thr, "bn4": b4t})

    import os
    trace = bool(os.environ.get("BNN_TRACE"))
    res = run_bass_kernel_spmd(nc, in_maps, core_ids=list(range(NCORES)),
                               trace=trace)
    if trace:
        _CACHE["last_exec_time_ns"] = res.exec_time_ns
        _CACHE["last_profile"] = res.profile_json

    # Gather: out [OUTP, NB] feature-major -> [B, OUT]
    y = np.empty((B, OUT), dtype=np.float32)
    for c in range(NCORES):
        y[c * NB:(c + 1) * NB] = res.results[c]["out"][:OUT, :].T

    _CACHE["last_y"] = y
    # npasses tail, replicating XLA's CSE'd graph bitwise:
    acc = y + y
    acc = acc + y
    acc = acc + y
    return acc * np.float32(0.25)


# revision 29
# speedup vs baseline: 1.1201x; 1.1201x over previous
"""Trainium2 Bass kernel for nn_Net_79027398246747 (4-layer binarized MLP).

Strategy (v2 — all-fp8 DoubleRow):
- Data-parallel over batch: 8 cores x 512 rows each; weights replicated.
- Feature-major internal layout (h.T [F, B]).
- Layer 1: x is decomposed on the host into FIVE radix-32 signed digits
  (24-bit fixed point of x, |x| < 8, LSB 2^-21, RNE at the bottom;
  X = sum_i d_i*32^i with d_i in [-15, 16] - every integer in that range
  has <= 4 significand bits, so d*2^s is exactly fp8e4m3-representable).
  Each digit array times the +-1 weights is an exact fp8 DoubleRow
  matmul. Digits 4,3 accumulate at true scale (2^-1, 2^-6) in PSUM group
  A; digits 2,1,0 at 2^14 x true scale (2^3, 2^-2, 2^-7) in group B
  (max value 16*2^3 = 128 < fp8e4m3's 240 ceiling; 2^-7 values are exact
  subnormal multiples of 2^-9; B's exact-accumulation bound verified:
  max |B| = 22k << 2^24 * 2^-7). The combine u = B*2^-14 + A costs one
  ACT + one DVE op. 5 passes at 0.5 cyc/row carry 24 bits - vs 6 passes
  for 4-bit nibbles and 2x fp32r passes (12-bit operands) at 1 cyc/row:
  2.5 total cycles/row/column for layer 1, the model's floor.
- Layers 2-4: activations +-0.5, weights +-1 -> exact fp8e4 DoubleRow.
- Epilogues are fused: BN gamma>0 and rsqrt>0, so sign(BN(p)) == (p >= T)
  with per-feature threshold T = m - b/(g*r) precomputed on the host
  (halved for layers fed by +-0.5 activations). One DVE op per f-tile:
  h = (p >= T) - 0.5; the missing 2x is folded into the next layer's
  threshold/affine. Layer 4 keeps the fused affine y = p*A + C.
- npasses=4 averaging: passes identical; replicate XLA's CSE'd tail
  (((y+y)+y)+y)*0.25 bitwise on host.
"""

import numpy as np
import ml_dtypes

B, IN, H, OUT = 4096, 3072, 4096, 1000
OUTP = 1024            # padded output features
NCORES = 8
NB = B // NCORES       # 512 batch rows per core
K1 = IN // 128         # 24 K-chunks for layer 1
K2 = H // 128          # 32 K-chunks for layers 2-4
F1 = H // 128          # 32 output feature tiles for layers 1-3
F4 = OUTP // 128       # 8 output feature tiles for layer 4
NI = 4                 # L1 f-tiles processed field-interleaved at the start
EPS = np.float32(1e-5)

_CACHE = {}


def _build_bass():
    import concourse.bacc as bacc
    import concourse.mybir as mybir
    from concourse.tile import TileContext

    fp32 = mybir.dt.float32
    fp8 = mybir.dt.float8e4
    DR = mybir.MatmulPerfMode.DoubleRow
    AO = mybir.AluOpType

    nc = bacc.Bacc(trn_type="TRN2")

    xf = nc.dram_tensor("xf", [5, 128, K1, NB], fp8, kind="ExternalInput")
    w1 = nc.dram_tensor("w1", [F1, 128, K1, 128], fp8, kind="ExternalInput")
    w2 = nc.dram_tensor("w2", [F1, 128, K2, 128], fp8, kind="ExternalInput")
    w3 = nc.dram_tensor("w3", [F1, 128, K2, 128], fp8, kind="ExternalInput")
    w4 = nc.dram_tensor("w4", [F4, 128, K2, 128], fp8, kind="ExternalInput")
    # Per-feature thresholds [128, 3, F1]: rows (T1, T2, T3), feature=f*128+p
    thr = nc.dram_tensor("thr", [128, 3, F1], fp32, kind="ExternalInput")
    bn4 = nc.dram_tensor("bn4", [128, 2, F4], fp32, kind="ExternalInput")
    out = nc.dram_tensor("out", [OUTP, NB], fp32, kind="ExternalOutput")

    with TileContext(nc) as tc:
        with tc.tile_pool(name="persist", bufs=1) as persist, \
             tc.tile_pool(name="w1pool", bufs=4) as w1pool, \
             tc.tile_pool(name="w23pool", bufs=8) as w23pool, \
             tc.tile_pool(name="zpool", bufs=3) as zpool, \
             tc.tile_pool(name="hpool", bufs=2) as hpool, \
             tc.tile_pool(name="psum", bufs=8, space="PSUM") as psum:

            xft = persist.tile([128, 5, K1, NB], fp8, name="xft")
            thrt = persist.tile([128, 3, F1], fp32)
            bnt4 = persist.tile([128, 2, F4], fp32)

            # --- DMA lead-in ---
            # x streams in 4-chunk pieces (728 ns transfer > the 625 ns
            # serialized HWDGE descriptor-gen, so the stream is not
            # HWDGE-bound). Order: w1[0], first 3 pieces, w1[1..3], rest.
            NP = 5 * K1 // 4                   # 30 pieces, 6 per field

            def dma_piece(q):
                i, t = divmod(q, 6)
                nc.sync.dma_start(out=xft[:, i, 4 * t:4 * t + 4, :],
                                  in_=xf[i, :, 4 * t:4 * t + 4, :])

            w1_pre = []
            w8_0 = w1pool.tile([128, K1, 128], fp8, tag="w1", name="w1_0")
            nc.sync.dma_start(out=w8_0[:], in_=w1[0])
            w1_pre.append(w8_0)
            for q in range(3):
                dma_piece(q)
            for f in range(1, NI):
                w8 = w1pool.tile([128, K1, 128], fp8, tag=f"w1p{f}",
                                 name=f"w1_{f}")
                nc.sync.dma_start(out=w8[:], in_=w1[f])
                w1_pre.append(w8)
            for q in range(3, NP):
                dma_piece(q)
            # needed first at the phase-A epilogues, well after the stream
            nc.sync.dma_start(out=thrt[:], in_=thr[:])
            nc.sync.dma_start(out=bnt4[:], in_=bn4[:])

            # h1 and h3 share a buffer (bufs=2 ring on one tag): h1 is fully
            # consumed by layer 2 before layer 3's epilogue writes h3.
            hs = [hpool.tile([128, F1, NB], fp8, tag="h", name=f"h{i}")
                  for i in range(3)]

            def epi1(psA, psB, f):
                # t = B*2^-14 (ACT, PSUM->SBUF); u = (A - T1) + t (DVE);
                # h = (u >= 0) - 0.5 (DVE)
                t = zpool.tile([128, NB], fp32, tag="t")
                nc.scalar.activation(t[:], psB[:],
                                     mybir.ActivationFunctionType.Copy,
                                     bias=0.0, scale=float(2.0 ** -14))
                u = zpool.tile([128, NB], fp32, tag="u")
                nc.vector.scalar_tensor_tensor(u[:], psA[:],
                                               thrt[:, 0, f:f + 1], t[:],
                                               op0=AO.subtract, op1=AO.add)
                nc.vector.tensor_scalar(hs[0][:, f, :], u[:], 0.0, 0.5,
                                        op0=AO.is_ge, op1=AO.subtract)

            # ---- Layer 1: 5 signed-digit fp8 DR passes, 2 PSUM groups ----
            # Phase A: first NI f-tiles run piece-interleaved so the PE
            # consumes each landed x piece NI times immediately. Emission
            # order tracks the DMA stream: pieces 0-2 tile-major (tile 0 can
            # start before w1[1..3] land), then piece-major for the rest.
            psA = [psum.tile([128, NB], fp32, tag="ps", name=f"psA{f}")
                   for f in range(NI)]
            psB = [psum.tile([128, NB], fp32, tag="ps", name=f"psB{f}")
                   for f in range(NI)]

            def l1mm(ps_pair, w8, q):
                # piece q = (field i, quarter t) -> 2 DR matmuls.
                # Group A = pieces 0..11 (digits 4,3), B = 12..29 (2,1,0).
                i, t = divmod(q, 6)
                ps = ps_pair[0] if q < 12 else ps_pair[1]
                for j in (2 * t, 2 * t + 1):
                    nc.tensor.matmul(ps[:], w8[:, 2 * j:2 * j + 2, :],
                                     xft[:, i, 2 * j:2 * j + 2, :],
                                     start=(q in (0, 12) and j == 2 * t),
                                     stop=(q in (11, 29) and j == 2 * t + 1),
                                     perf_mode=DR)

            for f in range(NI):
                for q in range(3):
                    l1mm((psA[f], psB[f]), w1_pre[f], q)
            for q in range(3, NP):
                for f in range(NI):
                    l1mm((psA[f], psB[f]), w1_pre[f], q)
            for f in range(NI):
                epi1(psA[f], psB[f], f)

            # Phase B: remaining f-tiles, sequential
            for f in range(NI, F1):
                w8 = w1pool.tile([128, K1, 128], fp8, tag="w1")
                nc.sync.dma_start(out=w8[:], in_=w1[f])
                pa = psum.tile([128, NB], fp32, tag="ps")
                pb = psum.tile([128, NB], fp32, tag="ps")
                for q in range(NP):
                    l1mm((pa, pb), w8, q)
                epi1(pa, pb, f)

            # Prefetch the first w4 tile early so layer 4 starts fed.
            w4pre = persist.tile([128, K2, 128], fp8, name="w4pre")
            nc.sync.dma_start(out=w4pre[:], in_=w4[0])

            # ---- Layers 2-3: fp8 DR, fused threshold epilogue ----
            for li, (w, hin, hout) in enumerate([(w2, hs[0], hs[1]),
                                                 (w3, hs[1], hs[2])], start=1):
                for f in range(F1):
                    w8 = w23pool.tile([128, K2, 128], fp8, tag="w23")
                    if f == 0:
                        nc.sync.dma_start(out=w8[:, :K2 // 2, :],
                                          in_=w[f, :, :K2 // 2, :])
                        nc.sync.dma_start(out=w8[:, K2 // 2:, :],
                                          in_=w[f, :, K2 // 2:, :])
                    else:
                        nc.sync.dma_start(out=w8[:], in_=w[f])
                    ps = psum.tile([128, NB], fp32, tag="ps")
                    for i in range(K2 // 2):
                        nc.tensor.matmul(ps[:], w8[:, 2 * i:2 * i + 2, :],
                                         hin[:, 2 * i:2 * i + 2, :],
                                         start=(i == 0),
                                         stop=(i == K2 // 2 - 1),
                                         perf_mode=DR)
                    nc.vector.tensor_scalar(hout[:, f, :], ps[:],
                                            thrt[:, li, f:f + 1], 0.5,
                                            op0=AO.is_ge, op1=AO.subtract)

            # ---- Layer 4: fp8 DR + fused affine y = p*A + C ----
            # The last tile runs as two half-batch PSUM groups so the final
            # epilogue + store overlap the closing matmuls.
            def epi4(ps, f, sl, eng=None):
                n = sl.stop - sl.start
                y = zpool.tile([128, n], fp32, tag="y")
                nc.vector.tensor_scalar(y[:], ps[:], bnt4[:, 0, f:f + 1],
                                        bnt4[:, 1, f:f + 1],
                                        op0=AO.mult, op1=AO.add)
                (eng or nc.sync).dma_start(out=out[f * 128:(f + 1) * 128, sl],
                                           in_=y[:])

            for f in range(F4):
                if f == 0:
                    w8 = w4pre
                else:
                    w8 = w23pool.tile([128, K2, 128], fp8, tag="w23",
                                      name=f"w4t{f}")
                    nc.sync.dma_start(out=w8[:], in_=w4[f])
                if f < F4 - 1:
                    ps = psum.tile([128, NB], fp32, tag="ps")
                    for i in range(K2 // 2):
                        nc.tensor.matmul(ps[:], w8[:, 2 * i:2 * i + 2, :],
                                         hs[2][:, 2 * i:2 * i + 2, :],
                                         start=(i == 0),
                                         stop=(i == K2 // 2 - 1),
                                         perf_mode=DR)
                    epi4(ps, f, slice(0, NB))
                else:
                    for half in range(2):
                        sl = slice(half * (NB // 2), (half + 1) * (NB // 2))
                        ph = psum.tile([128, NB // 2], fp32, tag="ps")
                        for i in range(K2 // 2):
                            nc.tensor.matmul(ph[:], w8[:, 2 * i:2 * i + 2, :],
                                             hs[2][:, 2 * i:2 * i + 2, sl],
                                             start=(i == 0),
                                             stop=(i == K2 // 2 - 1),
                                             perf_mode=DR)
                        epi4(ph, f, sl)

    nc.finalize()
    return nc


def _device_rsqrt(v):
    """rsqrt(v + eps) with the same bits as the neuron reference."""
    import jax
    fn = _CACHE.get("rsqrt_fn")
    if fn is None:
        fn = jax.jit(lambda t: jax.lax.rsqrt(t + EPS))
        _CACHE["rsqrt_fn"] = fn
    return np.asarray(fn(v.astype(np.float32)))


def _sign8(w):
    return np.where(w >= 0, 1, -1).astype(ml_dtypes.float8_e4m3)


def _prep_w(ws, n_k, n_f):
    # ws: [F_total, K_total] +-1 fp8 -> [n_f, 128, n_k, 128]:
    # out[f, p, k, j] = ws[f*128+j, k*128+p]
    a = ws.reshape(n_f, 128, n_k, 128)          # [f, j, k, p]
    return np.ascontiguousarray(a.transpose(0, 3, 2, 1))


def _threshold(bn, half):
    # bn: [4, F] (g, b, m, v); sign(BN(S)) == (S >= T), T = m - b/(g*r).
    # half: incoming activations are +-0.5 (preact = S/2) -> T/2.
    g, b, m, v = bn[0], bn[1], bn[2], bn[3]
    r = _device_rsqrt(v)
    T = (m - b / (g * r)).astype(np.float32)
    if half:
        T = (T * np.float32(0.5)).astype(np.float32)
    return T


def _digit_fields(x):
    # x [B, IN] fp32, |x| < 8 -> 5 stored-scale radix-32 signed-digit
    # arrays, fp8. X = rint(x * 2^21) = sum_i d_i * 32^i with d_i in
    # [-15, 16] (every such d*2^s is e4m3-exact). Array order is MSD
    # first: digit 4 (true 2^-1), 3 (2^-6) = PSUM group A; digits
    # 2, 1, 0 stored at 2^15 x true scale (2^4, 2^-1, 2^-6) = group B.
    X = np.rint(x * np.float32(2.0 ** 21)).astype(np.int64)
    X = np.clip(X, -(1 << 24) + 1, (1 << 24) - 1)  # |x|<8 guard (randn: never)
    digs = []
    R = X
    for i in range(5):
        r = R % 32                                 # in [0, 31]
        d = np.where(r <= 16, r, r - 32)
        digs.append(d)
        R = (R - d) >> 5
    assert not np.any(R), "radix-32 decomposition left a carry"
    stored = (2.0 ** -1, 2.0 ** -6, 2.0 ** 3, 2.0 ** -2, 2.0 ** -7)
    fields = []
    for i, dig in enumerate(reversed(digs)):       # MSD first
        f = dig.astype(np.float32) * np.float32(stored[i])
        fields.append(f.astype(ml_dtypes.float8_e4m3))
    return fields


def kernel(x, w1, w2, w3, w4, bn1, bn2, bn3, bn4):
    from concourse.bass_utils import run_bass_kernel_spmd

    x = np.asarray(x, dtype=np.float32)
    nc = _CACHE.get("nc")
    if nc is None:
        nc = _build_bass()
        _CACHE["nc"] = nc

    w1p = _prep_w(_sign8(np.asarray(w1)), K1, F1)
    w2p = _prep_w(_sign8(np.asarray(w2)), K2, F1)
    w3p = _prep_w(_sign8(np.asarray(w3)), K2, F1)
    w4s = _sign8(np.asarray(w4))
    w4pad = np.zeros((OUTP, H), dtype=ml_dtypes.float8_e4m3)
    w4pad[:OUT] = w4s
    w4p = _prep_w(w4pad, K2, F4)

    # thresholds [128, 3, F1]: thr[p, l, f] = T_l[f*128+p]
    T1 = _threshold(np.asarray(bn1), half=False)
    T2 = _threshold(np.asarray(bn2), half=True)
    T3 = _threshold(np.asarray(bn3), half=True)
    thr = np.stack([T1, T2, T3]).reshape(3, F1, 128)
    thr = np.ascontiguousarray(thr.transpose(2, 0, 1)).astype(np.float32)

    # layer-4 fused affine with the 0.5 fold: A = 2*g*r, C = b - (m*g)*r
    bn4a = np.asarray(bn4)
    g4 = np.zeros(OUTP, np.float32)
    b4 = np.zeros(OUTP, np.float32)
    m4 = np.zeros(OUTP, np.float32)
    v4 = np.full(OUTP, 1.0, np.float32)
    g4[:OUT], b4[:OUT], m4[:OUT], v4[:OUT] = bn4a[0], bn4a[1], bn4a[2], bn4a[3]
    r4 = _device_rsqrt(v4)
    A4 = (np.float32(2.0) * g4 * r4).astype(np.float32)
    C4 = (b4 - (m4 * g4) * r4).astype(np.float32)
    b4t = np.stack([A4, C4]).reshape(2, F4, 128)
    b4t = np.ascontiguousarray(b4t.transpose(2, 0, 1)).astype(np.float32)

    fields = _digit_fields(x)

    in_maps = []
    for c in range(NCORES):
        # field layout [128, K1, NB]: [p, k, n] = field[c*NB+n, k*128+p]
        xfc = np.empty((5, 128, K1, NB), dtype=ml_dtypes.float8_e4m3)
        for i in range(5):
            sl = fields[i][c * NB:(c + 1) * NB]           # [NB, IN]
            xfc[i] = sl.reshape(NB, K1, 128).transpose(2, 1, 0)
        in_maps.append({"xf": xfc, "w1": w1p, "w2": w2p, "w3": w3p,
                        "w4": w4p, "thr": thr, "bn4": b4t})

    import os
    trace = bool(os.environ.get("BNN_TRACE"))
    res = run_bass_kernel_spmd(nc, in_maps, core_ids=list(range(NCORES)),
                               trace=trace)
    if trace:
        _CACHE["last_exec_time_ns"] = res.exec_time_ns
        _CACHE["last_profile"] = res.profile_json

    # Gather: out [OUTP, NB] feature-major -> [B, OUT]
    y = np.empty((B, OUT), dtype=np.float32)
    for c in range(NCORES):
        y[c * NB:(c + 1) * NB] = res.results[c]["out"][:OUT, :].T

    _CACHE["last_y"] = y
    # npasses tail, replicating XLA's CSE'd graph bitwise:
    acc = y + y
    acc = acc + y
    acc = acc + y
    return acc * np.float32(0.25)
